# revision 5
# baseline (speedup 1.0000x reference)
"""LinOSS layer Trainium2 kernel, v3.

Math (same closed form as v1): the per-state 2x2 recurrence has eigenvalues
e^{+-i theta}; the scanned state collapses to rank-2 modulated prefix sums

    u     = s * Bu                     (s folded into B on host)
    E     = cumsum(T1 * u);  F = cumsum(T2 * u)     per complex part
    x_t   = sin(t th) * E_t + cos(t th) * F_t
    T1    = gamma*cos + sin;  T2 = cos - gamma*sin

v3 hardware structure (one core; states sharded 32/core, time folded 4x
into partitions -> [128 = 4 chunks x 32 states, 2048] tiles):
  - input is transposed ON THE HOST (inpT [H, L]); all device loads are
    plain async DMA streams.  (XBAR dma transposes gang all 16 DMA engines
    and stall ~7us whenever any other transfer is in flight.)
  - T1/T2/sinT/cosT tables built exactly on the host (f64 -> bf16)
  - modulation = scalar_tensor_tensor with accum_out: per-partition chunk
    sums are free; carry offsets (Wm matmul) feed the scans as initial
    values -> no post-scan bias pass
  - DVE chain: 4 stt mods, 4 scans, 4 muls + 2 adds (demod) — everything
    else stays off DVE (Pool is ~4x slower and halves DVE when co-run)
  - projection split into two slabs the host sums: out0 = Ctr@x_r during
    scans 3-4, out1 = Cti@x_i in the tail (32 matmuls of 512 cols total,
    the PE minimum for this contraction)
  - dD term dropped; host adds input*D exactly
"""

import numpy as np

L, H, P = 8192, 128, 256
NCORES = 8
SLOC = P // NCORES          # states per core
FOLD = 4                    # time chunks folded into partitions
CL = L // FOLD              # 2048 free columns per partition row
NPART = FOLD * SLOC         # 128
JT = 512                    # j-tile width (psum bank)
NJT = CL // JT              # 4

_CACHE: dict = {}


def _build_bass(split_waits=True):
    import concourse.bass as bass
    import concourse.mybir as mybir
    import concourse.tile as tile

    dt = mybir.dt.float32
    bt = mybir.dt.bfloat16
    Alu = mybir.AluOpType

    nc = bass.Bass(
        trn_type="TRN2",
        target_bir_lowering=False,
        debug=False,
        num_devices=NCORES,
    )

    inpT_d = nc.dram_tensor("inpT", [H, L], bt, kind="ExternalInput").ap()
    Bt_d = nc.dram_tensor("Bt", [H, 2 * SLOC], bt, kind="ExternalInput").ap()
    T1_d = nc.dram_tensor("T1", [NPART, CL], bt, kind="ExternalInput").ap()
    T2_d = nc.dram_tensor("T2", [NPART, CL], bt, kind="ExternalInput").ap()
    sin_d = nc.dram_tensor("sinT", [NPART, CL], bt, kind="ExternalInput").ap()
    cos_d = nc.dram_tensor("cosT", [NPART, CL], bt, kind="ExternalInput").ap()
    Ctr_d = nc.dram_tensor("Ctr", [NPART, H], bt, kind="ExternalInput").ap()
    Cti_d = nc.dram_tensor("Cti", [NPART, H], bt, kind="ExternalInput").ap()
    Wm_d = nc.dram_tensor("Wm", [NPART, NPART], dt, kind="ExternalInput").ap()
    out0 = nc.dram_tensor("out0", [H, L], bt, kind="ExternalOutput").ap()
    out1 = nc.dram_tensor("out1", [H, L], bt, kind="ExternalOutput").ap()

    with tile.TileContext(nc) as tc:
        cpool = tc.alloc_tile_pool(name="const", bufs=1)
        big = tc.alloc_tile_pool(name="big", bufs=1)
        stage = tc.alloc_tile_pool(name="stage", bufs=6)
        pbu_i_pool = tc.alloc_tile_pool(name="pbu_i", bufs=1, space="PSUM")
        pbu_r_pool = tc.alloc_tile_pool(name="pbu_r", bufs=1, space="PSUM")

        # ---- loads, earliest-needed first; all plain async streams ----
        Bt = cpool.tile([H, 2 * SLOC], bt)
        nc.sync.dma_start(out=Bt[:], in_=Bt_d)
        inpT = big.tile([128, L], bt, tag="inpT")
        for p8 in range(8):
            nc.sync.dma_start(
                out=inpT[:, p8 * (L // 8):(p8 + 1) * (L // 8)],
                in_=inpT_d[:, p8 * (L // 8):(p8 + 1) * (L // 8)],
            )
        # T1/T2 deferred until inpT is mostly in (DMA bw is fair-shared;
        # a WAR dep on a dummy slot is the only reliable deferral)
        T1d_t = big.tile([NPART, CL], bt, tag="T1")
        T2d_t = big.tile([NPART, CL], bt, tag="T2")
        gate1 = cpool.tile([1, 8], dt)
        nc.gpsimd.memset(T1d_t[0:1, 0:8], 0.0)
        nc.gpsimd.memset(T2d_t[0:1, 0:8], 0.0)
        nc.gpsimd.tensor_tensor(
            gate1[:], T1d_t[0:1, 0:8], inpT[0:1, 1 * (L // 8):1 * (L // 8) + 8],
            mybir.AluOpType.add)
        nc.gpsimd.tensor_tensor(
            gate1[:], T2d_t[0:1, 0:8], inpT[0:1, 1 * (L // 8):1 * (L // 8) + 8],
            mybir.AluOpType.add)
        T1 = big.tile([NPART, CL], bt, tag="T1")
        T2 = big.tile([NPART, CL], bt, tag="T2")
        for tt_, td_ in ((T1, T1_d), (T2, T2_d)):
            for hh in range(2):
                nc.sync.dma_start(
                    out=tt_[:, hh * (CL // 2):(hh + 1) * (CL // 2)],
                    in_=td_[:, hh * (CL // 2):(hh + 1) * (CL // 2)],
                )
        Ctr = cpool.tile([NPART, H], bt)
        Cti = cpool.tile([NPART, H], bt)
        Wm = cpool.tile([NPART, NPART], dt)
        nc.sync.dma_start(out=Ctr[:], in_=Ctr_d)
        nc.sync.dma_start(out=Cti[:], in_=Cti_d)
        nc.sync.dma_start(out=Wm[:], in_=Wm_d)

        ones = cpool.tile([NPART, CL], bt)
        nc.vector.memset(ones[:], 1.0)

        # ---- Bu matmuls into full-width PSUM; mods read PSUM directly
        # (stt has no 2x mode regardless, so the u evac would buy nothing)
        pbu_r = pbu_r_pool.tile([NPART, CL], dt, tag="bu_r")
        pbu_i = pbu_i_pool.tile([NPART, CL], dt, tag="bu_i")
        for pbu, bs in ((pbu_r, slice(0, SLOC)),
                        (pbu_i, slice(SLOC, 2 * SLOC))):
            for jt in range(NJT):
                for c in range(FOLD):
                    rhs = inpT[:, c * CL + jt * JT: c * CL + (jt + 1) * JT]
                    ps = slice(c * SLOC, (c + 1) * SLOC)
                    nc.tensor.matmul(
                        pbu[ps, jt * JT:(jt + 1) * JT], Bt[:, bs], rhs,
                        start=True, stop=True,
                        tile_position=(0, c * SLOC),
                    )

        # ---- deferred table loads: DMA bandwidth is fair-shared across all
        # in-flight transfers, so these 1.2MiB must not start until the
        # critical inpT/T1/T2 transfers are done.  A Pool op depending on
        # u_r gates the SWDGE issues. ----
        # sin/cos wave 3: gated on T2's arrival the same way
        sind_t = big.tile([NPART, CL], bt, tag="sinT")
        cosd_t = big.tile([NPART, CL], bt, tag="cosT")
        gatet = cpool.tile([1, 8], dt)
        nc.gpsimd.memset(sind_t[0:1, 0:8], 0.0)
        nc.gpsimd.memset(cosd_t[0:1, 0:8], 0.0)
        gsrc = inpT[0:1, 6 * (L // 8):6 * (L // 8) + 8]
        nc.gpsimd.tensor_tensor(
            gatet[:], sind_t[0:1, 0:8], gsrc, mybir.AluOpType.add)
        nc.gpsimd.tensor_tensor(
            gatet[:], cosd_t[0:1, 0:8], gsrc, mybir.AluOpType.add)
        sinT = big.tile([NPART, CL], bt, tag="sinT")
        cosT = big.tile([NPART, CL], bt, tag="cosT")
        # issue from two different engines so the transfers (and their
        # modeled arrivals) run in parallel: m_b needs cosT before scan-Ei
        nc.gpsimd.dma_start(out=sinT[:], in_=sin_d)
        nc.scalar.dma_start(out=cosT[:], in_=cos_d)

        # ---- modulation w/ fused chunk sums (DVE stt) ----
        A = cpool.tile([NPART, 4], dt)
        Y1r = big.tile([NPART, CL], bt, tag="Y1r")
        Y2r = big.tile([NPART, CL], bt, tag="Y2r")
        Y1i = big.tile([NPART, CL], bt, tag="Y1i")
        Y2i = big.tile([NPART, CL], bt, tag="Y2i")
        mods = [(Y1r, T1, pbu_r, 0), (Y2r, T2, pbu_r, 1),
                (Y1i, T1, pbu_i, 2), (Y2i, T2, pbu_i, 3)]
        offs = cpool.tile([NPART, 4], dt)
        for k, (Y, T, u, ai) in enumerate(mods):
            # modulation with fused chunk-sum accumulation (DVE stt)
            nc.vector.scalar_tensor_tensor(
                Y[:], T[:], 1.0, u[:], Alu.mult, Alu.mult,
                accum_out=A[:, ai:ai + 1],
            )
            if k == 1 or k == 3:
                # carry offsets for the pair just finished; output bounces
                # through a now-free corner of the pbu bank to SBUF so the
                # pbu pools can release before the projection needs PSUM
                cs = slice(0, 2) if k == 1 else slice(2, 4)
                pb = pbu_r if k == 1 else pbu_i
                nc.tensor.matmul(
                    pb[:, 0:2], Wm[:], A[:, cs],
                    start=True, stop=True,
                )
                nc.scalar.copy(offs[:, cs], pb[:, 0:2])

        pbu_r_pool.release()
        pbu_i_pool.release()
        po = tc.alloc_tile_pool(name="po", bufs=4, space="PSUM")

        # ---- scans (initial = carry offsets) + demod (all DVE) ----
        Er = big.tile([NPART, CL], bt, tag="Er")
        Fr = big.tile([NPART, CL], bt, tag="Fr")
        Ei = big.tile([NPART, CL], bt, tag="Ei")
        Fi = big.tile([NPART, CL], bt, tag="Fi")
        m_a = big.tile([NPART, CL], bt, tag="m_a")
        m_b = big.tile([NPART, CL], bt, tag="m_b")
        m_c = big.tile([NPART, CL], bt, tag="m_c")
        m_d = big.tile([NPART, CL], bt, tag="m_d")
        x_r = big.tile([NPART, CL], bt, tag="x_r")
        x_i = big.tile([NPART, CL], bt, tag="x_i")

        def scan(out, y, ai):
            bass.BassGpSimd.tensor_tensor_scan(
                nc.vector, out[:], ones[:], y[:], offs[:, ai:ai + 1],
                Alu.mult, Alu.add,
            )

        scan(Er, Y1r, 0)
        scan(Fr, Y2r, 1)
        with tc.high_priority():
            nc.vector.tensor_mul(m_a[:], Er[:], sinT[:])
            nc.vector.tensor_mul(m_b[:], Fr[:], cosT[:])
            nc.vector.tensor_add(x_r[:], m_a[:], m_b[:])
        scan(Ei, Y1i, 2)
        with tc.high_priority():
            nc.vector.tensor_mul(m_c[:], Ei[:], sinT[:])
        scan(Fi, Y2i, 3)
        with tc.high_priority():
            nc.vector.tensor_mul(m_d[:], Fi[:], cosT[:])
            nc.vector.tensor_add(x_i[:], m_c[:], m_d[:])

        # ---- projection slabs: out0 = Ctr@x_r (under scans 3-4),
        #      out1 = Cti@x_i (tail); host sums the slabs ----
        for slab, (Wt, x, outd) in enumerate(((Ctr, x_r, out0),
                                              (Cti, x_i, out1))):
            loop = ([(c, h2) for c in range(FOLD) for h2 in range(2)]
                    if slab == 0 else
                    [(c, h2) for h2 in range(2) for c in range(FOLD)])
            for c, h2 in loop:
                ps = slice(c * SLOC, (c + 1) * SLOC)
                if True:
                    pt = po.tile([128, 2 * JT], dt, tag="po")
                    for jh in range(2):
                        jt = 2 * h2 + jh
                        js = slice(jt * JT, (jt + 1) * JT)
                        nc.tensor.matmul(
                            pt[:, jh * JT:(jh + 1) * JT], Wt[ps, :],
                            x[ps, js], start=True, stop=True,
                            tile_position=(c * SLOC, 0),
                        )
                    st = stage.tile([128, 2 * JT], bt, tag="st")
                    # slab0 evacs run under the scans (ACT); slab1 evacs
                    # land in the tail, where DVE is free — alternate
                    if slab == 1 and (c * 2 + h2) % 2 == 0:
                        nc.vector.tensor_copy(st[:], pt[:])
                    else:
                        nc.scalar.copy(st[:], pt[:])
                    nc.sync.dma_start(
                        out=outd[:, c * CL + h2 * 2 * JT:
                                 c * CL + (h2 + 1) * 2 * JT],
                        in_=st[:],
                    )
        for p in (po, stage, big, cpool):
            p.release()
    if split_waits:
        _split_matmul_waits(nc, mybir)
    return nc


def _split_matmul_waits(nc, mybir):
    """Hardware instruction structs fit a limited number of embedded sync
    waits; move extra waits onto an inserted same-queue no-op."""
    caps = {"InstMatmult": 1}
    skip = {"InstNoOp", "InstAllEngineBarrier", "InstSync"}
    k = 0
    for bb in nc.main_func.blocks:
        insts = bb.instructions
        i = 0
        while i < len(insts):
            ins = insts[i]
            tn = type(ins).__name__
            if tn not in skip and ins.sync_info is not None:
                cap = caps.get(tn, 1)
                w = list(ins.sync_info.on_wait or [])
                if len(w) > cap:
                    for wj in w[:-cap]:
                        nop = mybir.InstNoOp(
                            name=f"I-mmdep-{k}",
                            engine=ins.engine,
                            ins=[],
                            outs=[],
                            sync_info=mybir.SyncInfo(
                                on_wait=[wj], on_update=[]
                            ),
                        )
                        k += 1
                        insts.insert(i, nop)
                        i += 1
                    ins.sync_info = mybir.SyncInfo(
                        on_wait=w[-cap:], on_update=ins.sync_info.on_update
                    )
            i += 1


def _host_prep(inputs):
    import ml_dtypes
    bf16 = ml_dtypes.bfloat16
    f32 = np.float32

    inp32 = np.asarray(inputs["input_sequence"], np.float32)
    inpT = np.ascontiguousarray(inp32.T.astype(bf16))
    A = np.maximum(np.asarray(inputs["A_diag_raw"], np.float64), 0.0)
    s = 1.0 / (1.0 + np.exp(-np.asarray(inputs["steps_raw"], np.float64)))
    Br = np.asarray(inputs["B_real"], np.float64)
    Bi = np.asarray(inputs["B_img"], np.float64)
    Cr = np.asarray(inputs["C_real"], np.float64)
    Ci = np.asarray(inputs["C_img"], np.float64)

    costh = 1.0 - s * s * A / 2.0
    sinth = np.sqrt(np.maximum(1.0 - costh * costh, 1e-300))
    theta = np.arctan2(sinth, costh)
    gamma = (s - s * s * A / 2.0) / sinth

    q = np.arange(NPART)
    Wm = ((q[:, None] % SLOC == q[None, :] % SLOC)
          & (q[:, None] // SLOC < q[None, :] // SLOC)).astype(f32)

    tvec = np.arange(CL, dtype=np.float64)
    twopi = 2.0 * np.pi

    in_maps = []
    for k in range(NCORES):
        sl = slice(k * SLOC, (k + 1) * SLOC)
        th = theta[sl]
        gm = gamma[sl]
        Bt = np.empty((H, 2 * SLOC), bf16)
        Bt[:, 0:SLOC] = (s[sl, None] * Br[sl]).T.astype(bf16)
        Bt[:, SLOC:] = (s[sl, None] * Bi[sl]).T.astype(bf16)
        Ctr = np.tile(Cr[:, sl].T, (FOLD, 1)).astype(bf16)
        Cti = np.tile(-Ci[:, sl].T, (FOLD, 1)).astype(bf16)

        # tables per partition q = c*SLOC + s at global time t = c*CL + j
        ang = np.empty((NPART, CL), np.float64)
        for c in range(FOLD):
            ang[c * SLOC:(c + 1) * SLOC] = np.mod(
                (c * CL + tvec)[None, :] * th[:, None], twopi)
        sinA = np.sin(ang)
        cosA = np.cos(ang)
        gq = np.tile(gm, FOLD)[:, None]
        T1 = (gq * cosA + sinA).astype(bf16)
        T2 = (cosA - gq * sinA).astype(bf16)

        in_maps.append({
            "inpT": inpT,
            "Bt": Bt,
            "T1": np.ascontiguousarray(T1),
            "T2": np.ascontiguousarray(T2),
            "sinT": np.ascontiguousarray(sinA.astype(bf16)),
            "cosT": np.ascontiguousarray(cosA.astype(bf16)),
            "Ctr": Ctr,
            "Cti": Cti,
            "Wm": Wm,
        })
    return in_maps


LAST_RESULTS = None


def kernel(**inputs) -> np.ndarray:
    global LAST_RESULTS
    from concourse.bass_utils import run_bass_kernel_spmd

    if "nc" not in _CACHE:
        _CACHE["nc"] = _build_bass()
    nc = _CACHE["nc"]

    in_maps = _host_prep(inputs)
    res = run_bass_kernel_spmd(nc, in_maps, core_ids=list(range(NCORES)))
    LAST_RESULTS = res
    part = np.zeros((H, L), np.float32)
    for r in res.results:
        part += np.asarray(r["out0"], np.float32)
        part += np.asarray(r["out1"], np.float32)
    out = np.ascontiguousarray(part.T)
    out += (np.asarray(inputs["input_sequence"], np.float32)
            * np.asarray(inputs["D"], np.float32)[None, :])
    return out


# revision 6
# speedup vs baseline: 1.0293x; 1.0293x over previous
"""LinOSS layer Trainium2 kernel, v3.

Math (same closed form as v1): the per-state 2x2 recurrence has eigenvalues
e^{+-i theta}; the scanned state collapses to rank-2 modulated prefix sums

    u     = s * Bu                     (s folded into B on host)
    E     = cumsum(T1 * u);  F = cumsum(T2 * u)     per complex part
    x_t   = sin(t th) * E_t + cos(t th) * F_t
    T1    = gamma*cos + sin;  T2 = cos - gamma*sin

v3 hardware structure (one core; states sharded 32/core, time folded 4x
into partitions -> [128 = 4 chunks x 32 states, 2048] tiles):
  - input is transposed ON THE HOST (inpT [H, L]); all device loads are
    plain async DMA streams.  (XBAR dma transposes gang all 16 DMA engines
    and stall ~7us whenever any other transfer is in flight.)
  - T1/T2/sinT/cosT tables built exactly on the host (f64 -> bf16)
  - modulation = scalar_tensor_tensor with accum_out: per-partition chunk
    sums are free; carry offsets (Wm matmul) feed the scans as initial
    values -> no post-scan bias pass
  - DVE chain: 4 stt mods, 4 scans, 4 muls + 2 adds (demod) — everything
    else stays off DVE (Pool is ~4x slower and halves DVE when co-run)
  - projection split into two slabs the host sums: out0 = Ctr@x_r during
    scans 3-4, out1 = Cti@x_i in the tail (32 matmuls of 512 cols total,
    the PE minimum for this contraction)
  - dD term dropped; host adds input*D exactly
"""

import numpy as np

L, H, P = 8192, 128, 256
NCORES = 8
SLOC = P // NCORES          # states per core
FOLD = 4                    # time chunks folded into partitions
CL = L // FOLD              # 2048 free columns per partition row
NPART = FOLD * SLOC         # 128
JT = 512                    # j-tile width (psum bank)
NJT = CL // JT              # 4

_CACHE: dict = {}


def _build_bass(split_waits=True):
    import concourse.bass as bass
    import concourse.mybir as mybir
    import concourse.tile as tile

    dt = mybir.dt.float32
    bt = mybir.dt.bfloat16
    Alu = mybir.AluOpType

    nc = bass.Bass(
        trn_type="TRN2",
        target_bir_lowering=False,
        debug=False,
        num_devices=NCORES,
    )

    inpT_d = nc.dram_tensor("inpT", [H, L], bt, kind="ExternalInput").ap()
    Bt_d = nc.dram_tensor("Bt", [H, 2 * SLOC], bt, kind="ExternalInput").ap()
    T1_d = nc.dram_tensor("T1", [NPART, CL], bt, kind="ExternalInput").ap()
    T2_d = nc.dram_tensor("T2", [NPART, CL], bt, kind="ExternalInput").ap()
    sin_d = nc.dram_tensor("sinT", [NPART, CL], bt, kind="ExternalInput").ap()
    cos_d = nc.dram_tensor("cosT", [NPART, CL], bt, kind="ExternalInput").ap()
    Ctr_d = nc.dram_tensor("Ctr", [NPART, H], bt, kind="ExternalInput").ap()
    Cti_d = nc.dram_tensor("Cti", [NPART, H], bt, kind="ExternalInput").ap()
    Wm_d = nc.dram_tensor("Wm", [NPART, NPART], dt, kind="ExternalInput").ap()
    out0 = nc.dram_tensor("out0", [H, L], bt, kind="ExternalOutput").ap()
    out1 = nc.dram_tensor("out1", [H, L], bt, kind="ExternalOutput").ap()

    with tile.TileContext(nc) as tc:
        cpool = tc.alloc_tile_pool(name="const", bufs=1)
        big = tc.alloc_tile_pool(name="big", bufs=1)
        stage = tc.alloc_tile_pool(name="stage", bufs=6)
        pbu_i_pool = tc.alloc_tile_pool(name="pbu_i", bufs=1, space="PSUM")
        pbu_r_pool = tc.alloc_tile_pool(name="pbu_r", bufs=1, space="PSUM")

        # ---- loads, earliest-needed first; all plain async streams ----
        Bt = cpool.tile([H, 2 * SLOC], bt)
        nc.sync.dma_start(out=Bt[:], in_=Bt_d)
        inpT = big.tile([128, L], bt, tag="inpT")
        for p8 in range(8):
            nc.sync.dma_start(
                out=inpT[:, p8 * (L // 8):(p8 + 1) * (L // 8)],
                in_=inpT_d[:, p8 * (L // 8):(p8 + 1) * (L // 8)],
            )
        # T1/T2 deferred until inpT is mostly in (DMA bw is fair-shared;
        # a WAR dep on a dummy slot is the only reliable deferral)
        T1d_t = big.tile([NPART, CL], bt, tag="T1")
        T2d_t = big.tile([NPART, CL], bt, tag="T2")
        gate1 = cpool.tile([1, 8], dt)
        nc.gpsimd.memset(T1d_t[0:1, 0:8], 0.0)
        nc.gpsimd.memset(T2d_t[0:1, 0:8], 0.0)
        nc.gpsimd.tensor_tensor(
            gate1[:], T1d_t[0:1, 0:8], inpT[0:1, 1 * (L // 8):1 * (L // 8) + 8],
            mybir.AluOpType.add)
        nc.gpsimd.tensor_tensor(
            gate1[:], T2d_t[0:1, 0:8], inpT[0:1, 1 * (L // 8):1 * (L // 8) + 8],
            mybir.AluOpType.add)
        T1 = big.tile([NPART, CL], bt, tag="T1")
        T2 = big.tile([NPART, CL], bt, tag="T2")
        for tt_, td_ in ((T1, T1_d), (T2, T2_d)):
            for hh in range(2):
                nc.sync.dma_start(
                    out=tt_[:, hh * (CL // 2):(hh + 1) * (CL // 2)],
                    in_=td_[:, hh * (CL // 2):(hh + 1) * (CL // 2)],
                )
        Ctr = cpool.tile([NPART, H], bt)
        Cti = cpool.tile([NPART, H], bt)
        Wm = cpool.tile([NPART, NPART], dt)
        nc.sync.dma_start(out=Ctr[:], in_=Ctr_d)
        nc.sync.dma_start(out=Cti[:], in_=Cti_d)
        nc.sync.dma_start(out=Wm[:], in_=Wm_d)

        ones = cpool.tile([NPART, CL], bt)
        nc.vector.memset(ones[:], 1.0)

        # ---- Bu matmuls into full-width PSUM; mods read PSUM directly
        # (stt has no 2x mode regardless, so the u evac would buy nothing)
        pbu_r = pbu_r_pool.tile([NPART, CL], dt, tag="bu_r")
        pbu_i = pbu_i_pool.tile([NPART, CL], dt, tag="bu_i")
        for pbu, bs in ((pbu_r, slice(0, SLOC)),
                        (pbu_i, slice(SLOC, 2 * SLOC))):
            for jt in range(NJT):
                for c in range(FOLD):
                    rhs = inpT[:, c * CL + jt * JT: c * CL + (jt + 1) * JT]
                    ps = slice(c * SLOC, (c + 1) * SLOC)
                    nc.tensor.matmul(
                        pbu[ps, jt * JT:(jt + 1) * JT], Bt[:, bs], rhs,
                        start=True, stop=True,
                        tile_position=(0, c * SLOC),
                    )

        # ---- deferred table loads: DMA bandwidth is fair-shared across all
        # in-flight transfers, so these 1.2MiB must not start until the
        # critical inpT/T1/T2 transfers are done.  A Pool op depending on
        # u_r gates the SWDGE issues. ----
        # sin/cos wave 3: gated on T2's arrival the same way
        sind_t = big.tile([NPART, CL], bt, tag="sinT")
        cosd_t = big.tile([NPART, CL], bt, tag="cosT")
        gatet = cpool.tile([1, 8], dt)
        nc.gpsimd.memset(sind_t[0:1, 0:8], 0.0)
        nc.gpsimd.memset(cosd_t[0:1, 0:8], 0.0)
        gsrc = inpT[0:1, 6 * (L // 8):6 * (L // 8) + 8]
        nc.gpsimd.tensor_tensor(
            gatet[:], sind_t[0:1, 0:8], gsrc, mybir.AluOpType.add)
        nc.gpsimd.tensor_tensor(
            gatet[:], cosd_t[0:1, 0:8], gsrc, mybir.AluOpType.add)
        sinT = big.tile([NPART, CL], bt, tag="sinT")
        cosT = big.tile([NPART, CL], bt, tag="cosT")
        # issue from two different engines so the transfers (and their
        # modeled arrivals) run in parallel: m_b needs cosT before scan-Ei
        nc.gpsimd.dma_start(out=sinT[:], in_=sin_d)
        nc.scalar.dma_start(out=cosT[:], in_=cos_d)

        # ---- modulation w/ fused chunk sums (DVE stt) ----
        A = cpool.tile([NPART, 4], dt)
        Y1r = big.tile([NPART, CL], bt, tag="Y1r")
        Y2r = big.tile([NPART, CL], bt, tag="Y2r")
        Y1i = big.tile([NPART, CL], bt, tag="Y1i")
        Y2i = big.tile([NPART, CL], bt, tag="Y2i")
        # T1-based mods first: T2 lands ~2.3us after T1, and this order
        # fills that DVE gap with Y1i instead of idling
        mods = [(Y1r, T1, pbu_r, 0), (Y1i, T1, pbu_i, 2),
                (Y2r, T2, pbu_r, 1), (Y2i, T2, pbu_i, 3)]
        offs = cpool.tile([NPART, 4], dt)
        for k, (Y, T, u, ai) in enumerate(mods):
            # modulation with fused chunk-sum accumulation (DVE stt)
            nc.vector.scalar_tensor_tensor(
                Y[:], T[:], 1.0, u[:], Alu.mult, Alu.mult,
                accum_out=A[:, ai:ai + 1],
            )
            if k == 2:
                # r-pair offsets (Y1r, Y2r read pbu_r; both done now, so
                # its corner is WAR-free for the matmul bounce)
                nc.tensor.matmul(
                    pbu_r[:, 0:2], Wm[:], A[:, 0:2], start=True, stop=True)
                nc.scalar.copy(offs[:, 0:2], pbu_r[:, 0:2])
            elif k == 3:
                nc.tensor.matmul(
                    pbu_i[:, 0:2], Wm[:], A[:, 2:4], start=True, stop=True)
                nc.scalar.copy(offs[:, 2:4], pbu_i[:, 0:2])

        pbu_r_pool.release()
        pbu_i_pool.release()
        po = tc.alloc_tile_pool(name="po", bufs=4, space="PSUM")

        # ---- scans (initial = carry offsets) + demod (all DVE) ----
        Er = big.tile([NPART, CL], bt, tag="Er")
        Fr = big.tile([NPART, CL], bt, tag="Fr")
        Ei = big.tile([NPART, CL], bt, tag="Ei")
        Fi = big.tile([NPART, CL], bt, tag="Fi")
        m_a = big.tile([NPART, CL], bt, tag="m_a")
        m_b = big.tile([NPART, CL], bt, tag="m_b")
        m_c = big.tile([NPART, CL], bt, tag="m_c")
        m_d = big.tile([NPART, CL], bt, tag="m_d")
        x_r = big.tile([NPART, CL], bt, tag="x_r")
        x_i = big.tile([NPART, CL], bt, tag="x_i")

        def scan(out, y, ai):
            bass.BassGpSimd.tensor_tensor_scan(
                nc.vector, out[:], ones[:], y[:], offs[:, ai:ai + 1],
                Alu.mult, Alu.add,
            )

        scan(Er, Y1r, 0)
        scan(Fr, Y2r, 1)
        with tc.high_priority():
            nc.vector.tensor_mul(m_a[:], Er[:], sinT[:])
            nc.vector.tensor_mul(m_b[:], Fr[:], cosT[:])
            nc.vector.tensor_add(x_r[:], m_a[:], m_b[:])
        scan(Ei, Y1i, 2)
        with tc.high_priority():
            nc.vector.tensor_mul(m_c[:], Ei[:], sinT[:])
        scan(Fi, Y2i, 3)
        with tc.high_priority():
            nc.vector.tensor_mul(m_d[:], Fi[:], cosT[:])
            nc.vector.tensor_add(x_i[:], m_c[:], m_d[:])

        # ---- projection slabs: out0 = Ctr@x_r (under scans 3-4),
        #      out1 = Cti@x_i (tail); host sums the slabs ----
        for slab, (Wt, x, outd) in enumerate(((Ctr, x_r, out0),
                                              (Cti, x_i, out1))):
            loop = ([(c, h2) for c in range(FOLD) for h2 in range(2)]
                    if slab == 0 else
                    [(c, h2) for h2 in range(2) for c in range(FOLD)])
            for c, h2 in loop:
                ps = slice(c * SLOC, (c + 1) * SLOC)
                if True:
                    pt = po.tile([128, 2 * JT], dt, tag="po")
                    for jh in range(2):
                        jt = 2 * h2 + jh
                        js = slice(jt * JT, (jt + 1) * JT)
                        nc.tensor.matmul(
                            pt[:, jh * JT:(jh + 1) * JT], Wt[ps, :],
                            x[ps, js], start=True, stop=True,
                            tile_position=(c * SLOC, 0),
                        )
                    st = stage.tile([128, 2 * JT], bt, tag="st")
                    # slab0 evacs run under the scans (ACT); slab1 evacs
                    # land in the tail, where DVE is free — alternate
                    if slab == 1 and (c * 2 + h2) % 2 == 0:
                        nc.vector.tensor_copy(st[:], pt[:])
                    else:
                        nc.scalar.copy(st[:], pt[:])
                    nc.sync.dma_start(
                        out=outd[:, c * CL + h2 * 2 * JT:
                                 c * CL + (h2 + 1) * 2 * JT],
                        in_=st[:],
                    )
        for p in (po, stage, big, cpool):
            p.release()
    if split_waits:
        _split_matmul_waits(nc, mybir)
    return nc


def _split_matmul_waits(nc, mybir):
    """Hardware instruction structs fit a limited number of embedded sync
    waits; move extra waits onto an inserted same-queue no-op."""
    caps = {"InstMatmult": 1}
    skip = {"InstNoOp", "InstAllEngineBarrier", "InstSync"}
    k = 0
    for bb in nc.main_func.blocks:
        insts = bb.instructions
        i = 0
        while i < len(insts):
            ins = insts[i]
            tn = type(ins).__name__
            if tn not in skip and ins.sync_info is not None:
                cap = caps.get(tn, 1)
                w = list(ins.sync_info.on_wait or [])
                if len(w) > cap:
                    for wj in w[:-cap]:
                        nop = mybir.InstNoOp(
                            name=f"I-mmdep-{k}",
                            engine=ins.engine,
                            ins=[],
                            outs=[],
                            sync_info=mybir.SyncInfo(
                                on_wait=[wj], on_update=[]
                            ),
                        )
                        k += 1
                        insts.insert(i, nop)
                        i += 1
                    ins.sync_info = mybir.SyncInfo(
                        on_wait=w[-cap:], on_update=ins.sync_info.on_update
                    )
            i += 1


def _host_prep(inputs):
    import ml_dtypes
    bf16 = ml_dtypes.bfloat16
    f32 = np.float32

    inp32 = np.asarray(inputs["input_sequence"], np.float32)
    inpT = np.ascontiguousarray(inp32.T.astype(bf16))
    A = np.maximum(np.asarray(inputs["A_diag_raw"], np.float64), 0.0)
    s = 1.0 / (1.0 + np.exp(-np.asarray(inputs["steps_raw"], np.float64)))
    Br = np.asarray(inputs["B_real"], np.float64)
    Bi = np.asarray(inputs["B_img"], np.float64)
    Cr = np.asarray(inputs["C_real"], np.float64)
    Ci = np.asarray(inputs["C_img"], np.float64)

    costh = 1.0 - s * s * A / 2.0
    sinth = np.sqrt(np.maximum(1.0 - costh * costh, 1e-300))
    theta = np.arctan2(sinth, costh)
    gamma = (s - s * s * A / 2.0) / sinth

    q = np.arange(NPART)
    Wm = ((q[:, None] % SLOC == q[None, :] % SLOC)
          & (q[:, None] // SLOC < q[None, :] // SLOC)).astype(f32)

    tvec = np.arange(CL, dtype=np.float64)
    twopi = 2.0 * np.pi

    in_maps = []
    for k in range(NCORES):
        sl = slice(k * SLOC, (k + 1) * SLOC)
        th = theta[sl]
        gm = gamma[sl]
        Bt = np.empty((H, 2 * SLOC), bf16)
        Bt[:, 0:SLOC] = (s[sl, None] * Br[sl]).T.astype(bf16)
        Bt[:, SLOC:] = (s[sl, None] * Bi[sl]).T.astype(bf16)
        Ctr = np.tile(Cr[:, sl].T, (FOLD, 1)).astype(bf16)
        Cti = np.tile(-Ci[:, sl].T, (FOLD, 1)).astype(bf16)

        # tables per partition q = c*SLOC + s at global time t = c*CL + j
        ang = np.empty((NPART, CL), np.float64)
        for c in range(FOLD):
            ang[c * SLOC:(c + 1) * SLOC] = np.mod(
                (c * CL + tvec)[None, :] * th[:, None], twopi)
        sinA = np.sin(ang)
        cosA = np.cos(ang)
        gq = np.tile(gm, FOLD)[:, None]
        T1 = (gq * cosA + sinA).astype(bf16)
        T2 = (cosA - gq * sinA).astype(bf16)

        in_maps.append({
            "inpT": inpT,
            "Bt": Bt,
            "T1": np.ascontiguousarray(T1),
            "T2": np.ascontiguousarray(T2),
            "sinT": np.ascontiguousarray(sinA.astype(bf16)),
            "cosT": np.ascontiguousarray(cosA.astype(bf16)),
            "Ctr": Ctr,
            "Cti": Cti,
            "Wm": Wm,
        })
    return in_maps


LAST_RESULTS = None


def kernel(**inputs) -> np.ndarray:
    global LAST_RESULTS
    from concourse.bass_utils import run_bass_kernel_spmd

    if "nc" not in _CACHE:
        _CACHE["nc"] = _build_bass()
    nc = _CACHE["nc"]

    in_maps = _host_prep(inputs)
    res = run_bass_kernel_spmd(nc, in_maps, core_ids=list(range(NCORES)))
    LAST_RESULTS = res
    part = np.zeros((H, L), np.float32)
    for r in res.results:
        part += np.asarray(r["out0"], np.float32)
        part += np.asarray(r["out1"], np.float32)
    out = np.ascontiguousarray(part.T)
    out += (np.asarray(inputs["input_sequence"], np.float32)
            * np.asarray(inputs["D"], np.float32)[None, :])
    return out
